# revision 1
# baseline (speedup 1.0000x reference)
"""Trainium2 Bass kernel for BilinearClassification (segment_reduce).

Math (per example b):
  ent[e,:]  = masked-mean over subword span of hidden[idx[e,s],:]      (E=64, H=768)
  subj[t,:] = ent[trip[t,0],:] * pm[t];  obj[t,:] = ent[trip[t,1],:] * pm[t]
  bl[t, (g,i,j)] = subj[t, g*8+i] * obj[t, g*8+j]                      (f = 6144)
  logits[t,n] = bl[t,:] @ W[:,n] + b[n]                                (NT=42)

Device strategy (8 cores, 4 examples each, no collectives, all-bf16 inputs):
  - host precomputes one-hot segment-mean matrix AT[l, (ex,e)] (mask, 1/cnt and
    example-pair block-diagonal folded) and pair-stacked triplet one-hots
    G_s/G_o [128=(2ex x 64e), 256=(2ex x 128t)] (pair_mask folded), so the
    device only does matmuls + copies + one elementwise mul.
  - stage 1: ent = AT.T @ hidden; two 384-col PSUM halves per example pair.
  - tables: ent_exp_s[(ex,e), (g,i,j)] = ent[(ex,e), 8g+i] (and _o with 8g+j),
    built by broadcast-AP copies (ACT/DVE for the first slices, idle GPSIMD
    for the rest) because matmul weights APs must be 2D.
  - stage 2 per f-chunk c (128 f-rows = 2 bilinear groups x 8i x 8j):
      S_exp = ent_exp_s[:, c-slice].T @ G_s   -> PSUM [128, 256]
      O_exp = ent_exp_o[:, c-slice].T @ G_o   -> PSUM [128, 256]
      s_sb  = ACT evac of S_exp; blT_c = s_sb * O_exp (DVE) -> SBUF bf16
      logits += W_c.T @ blT_c                 (PSUM accumulate over 48 chunks)
  - host adds b_fc and reshapes.
"""
import sys

sys.path.insert(0, "/opt/trn_rl_repo")

import numpy as np

import concourse.bass as bass
import concourse.bacc as bacc
import concourse.tile as tile
from concourse import mybir
from concourse.bass_utils import run_bass_kernel_spmd

F32 = mybir.dt.float32
BF16 = mybir.dt.bfloat16

B, L, H = 32, 512, 768
E, S, T = 64, 8, 128
NT = 42
NCORES = 8
EXPC = B // NCORES          # 4 examples per core
NPAIR = EXPC // 2           # 2 example-pairs per core
KC = L // 128               # 4 contraction chunks over l
FC = (H * 8) // 128         # 48 f-chunks
TP = 2 * T                  # 256 columns per pair (2ex x 128t)
NH = 2                      # ent psum halves (384 cols each)
HHALF = H // NH


def build_program(reps=1):
    """reps>1 repeats the whole body back-to-back (for wall-clock timing
    amplification in the test harness; the grading path uses reps=1)."""
    nc = bacc.Bacc("TRN2", target_bir_lowering=False, debug=False)

    hid_d = nc.dram_tensor("hid", (EXPC * L, H), BF16, kind="ExternalInput")
    # AT is pair-block-diagonal: rows (ex,kc,l), cols (ex' * 64 + e)
    at_d = nc.dram_tensor("at", (EXPC * L, 2 * E), BF16, kind="ExternalInput")
    gs_d = nc.dram_tensor("gs", (NPAIR, 128, TP), BF16, kind="ExternalInput")
    go_d = nc.dram_tensor("go", (NPAIR, 128, TP), BF16, kind="ExternalInput")
    # W pre-shuffled on host to the SBUF chunk layout [p, (c n)]
    w_d = nc.dram_tensor("w", (128, FC * NT), BF16, kind="ExternalInput")
    out_d = nc.dram_tensor("out", (NPAIR, NT, TP), F32, kind="ExternalOutput")

    with tile.TileContext(nc) as tc:
        with (
            tc.tile_pool(name="consts", bufs=1) as consts,
            tc.tile_pool(name="hidp", bufs=4) as hidp,
            tc.tile_pool(name="atp", bufs=2) as atp,
            tc.tile_pool(name="entps", bufs=2, space="PSUM") as entps,
            tc.tile_pool(name="entsb", bufs=2) as entsb,
            tc.tile_pool(name="tabp", bufs=2) as tabp,
            tc.tile_pool(name="sops", bufs=2, space="PSUM") as sops,
            tc.tile_pool(name="lgps", bufs=2, space="PSUM") as lgps,
            tc.tile_pool(name="blp", bufs=6) as blp,
            tc.tile_pool(name="outp", bufs=2) as outp,
        ):
          for _rep in range(reps):
            # ---- input DMAs, consolidated (HWDGE setup is ~600ns per DMA)
            # and ordered so pair 0's ent inputs land first
            hid_t = [None] * NPAIR   # [128, 8ck, H] per pair
            at_t = [None] * NPAIR    # [128, 8ck, 2E] per pair
            gs_t = [None] * NPAIR
            go_t = [None] * NPAIR
            w_all = None
            for P in range(NPAIR):
                att = atp.tile([128, 2 * KC, 2 * E], BF16)
                at_r = at_d[:].rearrange("(ck p) e -> p ck e", p=128)
                nc.sync.dma_start(att[:], at_r[:, P * 2 * KC : (P + 1) * 2 * KC, :])
                at_t[P] = att
                hid_r = hid_d[:].rearrange("(ck p) h -> p ck h", p=128)
                hts = []
                for half in range(2):  # separate tiles so deps are per-half
                    h1 = hidp.tile([128, KC, H], BF16)
                    if P == 0 and half == 0:
                        # finest granularity on the critical first chunks so
                        # the very first ent matmul starts as early as possible
                        for ck in range(KC):
                            nc.sync.dma_start(
                                h1[:, ck, :], hid_r[:, P * 2 * KC + ck, :])
                    else:
                        nc.sync.dma_start(
                            h1[:],
                            hid_r[:, P * 2 * KC + half * KC : P * 2 * KC + (half + 1) * KC, :])
                    hts.append(h1)
                hid_t[P] = hts
                g1 = consts.tile([128, TP], BF16, tag=f"gs{P}")
                nc.sync.dma_start(g1[:], gs_d[P])
                gs_t[P] = g1
                g2 = consts.tile([128, TP], BF16, tag=f"go{P}")
                nc.sync.dma_start(g2[:], go_d[P])
                go_t[P] = g2
                if P == 0:
                    w_all = consts.tile([128, FC, NT], BF16, tag="w")
                    nc.sync.dma_start(
                        w_all[:], w_d[:].rearrange("p (c n) -> p c n", n=NT))

            # ---- stage 1 + tables for every pair first (program order)
            tabs = []
            for P in range(NPAIR):
                ent_sb = entsb.tile([128, H], BF16, tag="ent_sb")
                tab_s = tabp.tile([128, H * 8], BF16, tag="tab_s")
                tab_o = tabp.tile([128, H * 8], BF16, tag="tab_o")
                for nh in range(NH):
                    fast = P == 0 and nh == 0
                    ent_ps = entps.tile([128, HHALF], F32)
                    for ck in range(2 * KC):
                        nc.tensor.matmul(
                            ent_ps[:],
                            at_t[P][:, ck, :],
                            hid_t[P][ck // KC][:, ck % KC,
                                               nh * HHALF : (nh + 1) * HHALF],
                            start=(ck == 0),
                            stop=(ck == 2 * KC - 1),
                        )
                    if not fast:
                        # GPSIMD has no PSUM port: stage this half into SBUF
                        nc.scalar.copy(
                            ent_sb[:, nh * HHALF : (nh + 1) * HHALF], ent_ps[:])
                    # table slices covering this half: groups [nh*48, (nh+1)*48)
                    g0 = nh * (96 // NH)
                    gn2 = 96 // NH // 2   # two slices per half
                    for sl in range(2):
                        ga = g0 + sl * gn2
                        if fast:
                            # straight from PSUM on the idle fast engines so
                            # pair 0 stage 2 starts as early as possible
                            src = ent_ps[:, (ga - g0) * 8 : (ga - g0 + gn2) * 8]
                        else:
                            src = ent_sb[:, ga * 8 : (ga + gn2) * 8]
                        src_s = (src.rearrange("p (g i) -> p g i", i=8)
                                 .unsqueeze(3).broadcast_to((128, gn2, 8, 8)))
                        src_o = (src.rearrange("p (g j) -> p g j", j=8)
                                 .unsqueeze(2).broadcast_to((128, gn2, 8, 8)))
                        dst_s = tab_s[:, ga * 64 : (ga + gn2) * 64].rearrange(
                            "p (g i j) -> p g i j", i=8, j=8)
                        dst_o = tab_o[:, ga * 64 : (ga + gn2) * 64].rearrange(
                            "p (g i j) -> p g i j", i=8, j=8)
                        if fast:
                            nc.scalar.copy(dst_s, src_s)
                            nc.vector.tensor_copy(dst_o, src_o)
                        else:
                            # on GPSIMD, overlapped with running stage 2
                            nc.gpsimd.tensor_copy(dst_s, src_s)
                            nc.gpsimd.tensor_copy(dst_o, src_o)
                tabs.append((tab_s, tab_o))

            # ---- stage 2: 48 f-chunks per pair, merged in pairs of chunks
            for P in range(NPAIR):
                tab_s, tab_o = tabs[P]
                lg_ps = lgps.tile([NT, TP], F32)
                for cc in range(FC // 2):
                    s_ps = sops.tile([128, 2, TP], F32, tag="s")
                    o_ps = sops.tile([128, 2, TP], F32, tag="o")
                    for h in range(2):
                        c = cc * 2 + h
                        nc.tensor.matmul(
                            s_ps[:, h, :],
                            tab_s[:, c * 128 : (c + 1) * 128],
                            gs_t[P][:],
                            start=True,
                            stop=True,
                        )
                        nc.tensor.matmul(
                            o_ps[:, h, :],
                            tab_o[:, c * 128 : (c + 1) * 128],
                            go_t[P][:],
                            start=True,
                            stop=True,
                        )
                    # DVE can read at most one PSUM operand: evacuate S via ACT
                    s_sb = blp.tile([128, 2, TP], F32, tag="s_sb")
                    nc.scalar.copy(s_sb[:], s_ps[:])
                    blt = blp.tile([128, 2, TP], BF16, tag="blt")
                    nc.vector.tensor_mul(blt[:], s_sb[:], o_ps[:])
                    for h in range(2):
                        c = cc * 2 + h
                        nc.tensor.matmul(
                            lg_ps[:],
                            w_all[:, c, :],
                            blt[:, h, :],
                            start=(c == 0),
                            stop=(c == FC - 1),
                        )

                out_sb = outp.tile([NT, TP], F32)
                nc.scalar.copy(out_sb[:], lg_ps[:])
                nc.sync.dma_start(out_d[P], out_sb[:])

    nc.compile()
    return nc


def host_prep(hidden_states, entity_subw_indices, entity_subw_mask,
              triplet_entity_nums, pair_mask, W_fc):
    """Build per-core input maps (numpy only, cheap)."""
    import ml_dtypes
    bf16 = ml_dtypes.bfloat16
    hs = np.asarray(hidden_states, dtype=np.float32).astype(bf16)
    idx = np.asarray(entity_subw_indices)
    msk = np.asarray(entity_subw_mask).astype(np.float32)
    trip = np.asarray(triplet_entity_nums)
    pm = np.asarray(pair_mask).astype(np.float32)
    # shuffle W to the SBUF chunk layout [p, (c, n)]
    w = (np.asarray(W_fc, dtype=np.float32).reshape(FC, 128, NT)
         .transpose(1, 0, 2).reshape(128, FC * NT).astype(bf16))

    # AT[b]: (L, 2E) pair-block-diagonal with mask/cnt folded
    cnt = np.maximum(msk.sum(axis=2), 1.0)          # (B, E)
    wgt = msk / cnt[:, :, None]                     # (B, E, S)
    at = np.zeros((B, L, 2 * E), np.float32)
    b_i, e_i, s_i = np.nonzero(msk > 0)
    np.add.at(at, (b_i, idx[b_i, e_i, s_i], (b_i % 2) * E + e_i),
              wgt[b_i, e_i, s_i])
    at = at.astype(bf16)

    # pair-stacked block-diagonal triplet one-hots (2ex x 64e, 2ex x 128t)
    gs = np.zeros((B // 2, 128, TP), bf16)
    go = np.zeros((B // 2, 128, TP), bf16)
    bb = np.arange(B)[:, None]
    tt = np.arange(T)[None, :]
    pair = bb // 2
    exl = (bb % 2)
    gs[pair, exl * E + trip[:, :, 0], exl * T + tt] = pm.astype(bf16)
    go[pair, exl * E + trip[:, :, 1], exl * T + tt] = pm.astype(bf16)

    in_maps = []
    for c in range(NCORES):
        b0 = c * EXPC
        in_maps.append({
            "hid": np.ascontiguousarray(hs[b0 : b0 + EXPC].reshape(EXPC * L, H)),
            "at": np.ascontiguousarray(
                at[b0 : b0 + EXPC].reshape(EXPC * L, 2 * E)),
            "gs": np.ascontiguousarray(gs[b0 // 2 : b0 // 2 + NPAIR]),
            "go": np.ascontiguousarray(go[b0 // 2 : b0 // 2 + NPAIR]),
            "w": w,
        })
    return in_maps


def assemble(results, b_fc):
    """results[c]["out"] is (NPAIR, NT, 2ex x 128t) -> (B, T, NT) + bias."""
    logits = np.empty((B, T, NT), np.float32)
    for c in range(NCORES):
        o = results[c]["out"].reshape(NPAIR, NT, 2, T)
        for P in range(NPAIR):
            for exl in range(2):
                b = c * EXPC + P * 2 + exl
                logits[b] = o[P, :, exl, :].T
    return logits + np.asarray(b_fc, np.float32)[None, None, :]


_NC_CACHE = None


def kernel(hidden_states, entity_subw_indices, entity_subw_mask,
           triplet_entity_nums, pair_mask, W_fc, b_fc):
    global _NC_CACHE
    if _NC_CACHE is None:
        _NC_CACHE = build_program()
    nc = _NC_CACHE
    in_maps = host_prep(hidden_states, entity_subw_indices, entity_subw_mask,
                        triplet_entity_nums, pair_mask, W_fc)
    res = run_bass_kernel_spmd(nc, in_maps, core_ids=list(range(NCORES)))
    return assemble(res.results, b_fc)



# revision 18
# speedup vs baseline: 1.3562x; 1.3562x over previous
"""Trainium2 Bass kernel for BilinearClassification (segment_reduce).

Math (per example b):
  ent[e,:]  = masked-mean over subword span of hidden[idx[e,s],:]      (E=64, H=768)
  subj[t,:] = ent[trip[t,0],:];  obj[t,:] = ent[trip[t,1],:]           (valid t only)
  bl[t, (g,i,j)] = subj[t, 8g+i] * obj[t, 8g+j]                        (f = 6144)
  logits[t,n] = bl[t,:] @ W[:,n] + b[n]                                (NT=42)

Device strategy (8 cores, data parallel, no collectives):
  - Host balances examples across cores by valid-triplet count, packs only the
    VALID triplet columns (T_PACK ~= 272 instead of 512) and only the USED
    hidden rows per example pair (RPP ~= 768 instead of 1024), and permutes
    hidden columns to (i,g) order: col' = i*96 + g for h = 8g+i.
  - stage 1: ent[e, (i,g)] = AT.T @ hid per pair (one-hot AT with mask/1/cnt
    folded), PSUM halves, ACT evac to bf16.
  - gathers: compact S_i[g,t] = ent[:, i*96:(i+1)*96].T @ G_s (accumulated over
    the core's two pairs), likewise O_j.  Thanks to the (i,g) column
    permutation these are plain 2D weight slices.  32 matmuls of T_PACK cols.
  - bl_i[g, j, t] = S_i[g, t] * O_j[g, t]: ONE broadcast-AP DVE multiply per i
    (all-SBUF bf16 -> 2x DVE rate).  No 6144-row expansion matmuls and no
    expanded-operand PSUM evacuations anywhere.
  - final: logits[n, t] += w3[:, i, j, :].T @ bl_i[:, j, :], 64 accumulating
    matmuls into one PSUM bank (42 x T_PACK).
  - host scatters packed columns back to (b, t) and adds b_fc.
"""
import sys

sys.path.insert(0, "/opt/trn_rl_repo")

import numpy as np

import concourse.bass as bass
import concourse.bacc as bacc
import concourse.tile as tile
from concourse import mybir
from concourse.bass_utils import run_bass_kernel_spmd

F32 = mybir.dt.float32
BF16 = mybir.dt.bfloat16

B, L, H = 32, 512, 768
E, S, T = 64, 8, 128
NT = 42
NCORES = 8
EXPC = B // NCORES          # 4 examples per core
NPAIR = EXPC // 2           # 2 example-pairs per core
GRP = 96                    # bilinear groups
NI = 8                      # i (subj) positions per group
NJ = 8                      # j (obj) positions per group
HHALF = H // 2

# hidden column permutation: col' = i*96 + g  <->  h = 8g + i
HPERM = np.empty(H, np.int64)
for _i in range(NI):
    for _g in range(GRP):
        HPERM[_i * GRP + _g] = 8 * _g + _i


class Plan:
    """Data-dependent packing decisions (shapes are compile-time params)."""

    def __init__(self, entity_subw_indices, entity_subw_mask,
                 triplet_entity_nums, pair_mask):
        idx = np.asarray(entity_subw_indices)
        msk = np.asarray(entity_subw_mask).astype(bool)
        pm = np.asarray(pair_mask).astype(bool)
        ntrip = pm.sum(axis=1).astype(int)

        # balance examples over cores by triplet count: greedy + swap
        # local search (TP <= 256 lets two gather tiles share a PSUM bank)
        order = np.argsort(-ntrip, kind="stable")
        cores = [[] for _ in range(NCORES)]
        loads = [0] * NCORES
        for ex in order:
            c = min((c for c in range(NCORES) if len(cores[c]) < EXPC),
                    key=lambda c: loads[c])
            cores[c].append(int(ex))
            loads[c] += int(ntrip[ex])
        for _ in range(1000):
            cmax = int(np.argmax(loads))
            best = None
            for c2 in range(NCORES):
                if c2 == cmax:
                    continue
                for ia, a in enumerate(cores[cmax]):
                    for ib, b in enumerate(cores[c2]):
                        d = int(ntrip[a]) - int(ntrip[b])
                        if d <= 0:
                            continue
                        nm = max(loads[cmax] - d, loads[c2] + d)
                        if nm < loads[cmax] and (best is None or nm < best[0]):
                            best = (nm, c2, ia, ib)
            if best is None:
                break
            _, c2, ia, ib = best
            a, b = cores[cmax][ia], cores[c2][ib]
            cores[cmax][ia], cores[c2][ib] = b, a
            d = int(ntrip[a]) - int(ntrip[b])
            loads[cmax] -= d
            loads[c2] += d
        self.cores = cores
        self.TP = max(16, -(-max(loads) // 16) * 16)

        # rows actually used per example; pair big-with-small inside a core
        self.rows = [np.unique(idx[b][msk[b]]) for b in range(B)]
        uniq = [len(r) for r in self.rows]
        self.pairs = []            # per core: [(exA, exB), (exC, exD)]
        rpp = 1
        for c in range(NCORES):
            exs = sorted(cores[c], key=lambda b: -uniq[b])
            ps = [(exs[0], exs[3]), (exs[1], exs[2])]
            self.pairs.append(ps)
            for a, b2 in ps:
                rpp = max(rpp, uniq[a] + uniq[b2])
        self.RPP = -(-rpp // 128) * 128
        self.CKP = self.RPP // 128

        # packed triplet columns per core: (example, orig t)
        self.cols = []
        for c in range(NCORES):
            cc = []
            for b in cores[c]:
                for t in np.nonzero(pm[b])[0]:
                    cc.append((b, int(t)))
            self.cols.append(cc)

        # example -> (pair index, local index) within its core
        self.exloc = {}
        for c in range(NCORES):
            for P, (a, b2) in enumerate(self.pairs[c]):
                self.exloc[a] = (c, P, 0)
                self.exloc[b2] = (c, P, 1)


POOL_MULT = (5, 6, 7)     # i-slices whose bl multiply runs on Pool, not DVE
ENT_EVAC = ("v", "a", "v", "a")      # per (half, pair) evac engine
O_EVAC = ("a", "v", "a", "v")    # per O slice-pair (O01, O23, O45, O67)
FINAL_T = True            # transposed final matmuls (stream W, bl stationary)


def build_program(TP, CKP, reps=1, final_t=None):
    if final_t is None:
        final_t = FINAL_T
    RPP = CKP * 128
    nc = bacc.Bacc("TRN2", target_bir_lowering=False, debug=False)

    # t-chunks (<=128 wide) for the transposed final mode
    tcs = []
    off = 0
    while off < TP:
        tcs.append((off, min(128, TP - off)))
        off += 128

    hid_d = nc.dram_tensor("hid", (NPAIR * RPP, H), BF16, kind="ExternalInput")
    at_d = nc.dram_tensor("at", (NPAIR * RPP, 2 * E), BF16, kind="ExternalInput")
    # g: [go0, go1, gs0, gs1] stacked -> one DMA
    g_d = nc.dram_tensor("g", (2 * NPAIR, 128, TP), BF16, kind="ExternalInput")
    w_d = nc.dram_tensor("w", (GRP, NI * NJ * NT), BF16, kind="ExternalInput")
    if final_t:
        out_d = nc.dram_tensor("out", (len(tcs) * 128, NT), F32,
                               kind="ExternalOutput")
    else:
        out_d = nc.dram_tensor("out", (NT, TP), F32, kind="ExternalOutput")

    hid_r = hid_d[:].rearrange("(q p) h -> p q h", p=128)   # q = pair*CKP + ck
    at_r = at_d[:].rearrange("(q p) e -> p q e", p=128)
    g_r = g_d[:].rearrange("k p t -> p k t")

    # gather-psum group geometry: two [GRP, TP] gathers share a group tile;
    # SH is the f32 stride between them (one bank when 2*TP*4 <= 2KB)
    SH = TP if TP <= 256 else 512
    if TP <= 256:
        gps_bufs, eps_bufs = 3, 3
    else:
        gps_bufs, eps_bufs = 2, (1 if final_t else 2)

    with tile.TileContext(nc) as tc:
        with (
            tc.tile_pool(name="consts", bufs=1) as consts,
            tc.tile_pool(name="hidp", bufs=2) as hidp,
            tc.tile_pool(name="atp", bufs=2) as atp,
            tc.tile_pool(name="entps", bufs=eps_bufs, space="PSUM") as entps,
            tc.tile_pool(name="entsb", bufs=2) as entsb,
            tc.tile_pool(name="gps", bufs=gps_bufs, space="PSUM") as gps,
            tc.tile_pool(name="osb", bufs=1) as osb,
            tc.tile_pool(name="ssb", bufs=4) as ssb,
            tc.tile_pool(name="blp", bufs=NI) as blp,
            tc.tile_pool(name="lgps", bufs=1, space="PSUM") as lgps,
            tc.tile_pool(name="outp", bufs=2) as outp,
        ):
          for _rep in range(reps):
            # ---- input DMAs spread over the three DMA-capable queues
            # (SP, Pool/SWDGE, ACT).  The issuing queue is blocked for the
            # whole transfer, so hid is chunk-split across all three; ACT
            # only carries one small chunk per pair so it is free for evacs.
            hm = CKP // 2
            at_t, hid_t = [], []
            for P in range(NPAIR):
                att = atp.tile([128, CKP, 2 * E], BF16)
                at_t.append(att)
                ht = hidp.tile([128, CKP, H], BF16)
                hid_t.append(ht)
            nc.sync.dma_start(at_t[0][:], at_r[:, 0:CKP, :])
            nc.gpsimd.dma_start(at_t[1][:], at_r[:, CKP:2 * CKP, :])
            for ck in range(CKP):
                # round-robin SP / Pool / ACT, pair 0 chunks first
                for P in range(NPAIR):
                    eng = (nc.sync, nc.gpsimd, nc.scalar)[(ck * NPAIR + P) % 3]
                    eng.dma_start(hid_t[P][:, ck, :], hid_r[:, P * CKP + ck, :])
            g_t = consts.tile([128, 2 * NPAIR, TP], BF16, tag="g")
            nc.sync.dma_start(g_t[:], g_r[:])
            go_t = [g_t[:, 0, :], g_t[:, 1, :]]
            gs_t = [g_t[:, 2, :], g_t[:, 3, :]]
            w_t = consts.tile([GRP, NI * NJ * NT], BF16, tag="w")
            nc.sync.dma_start(w_t[:], w_d[:])

            def evac(tag, dst, src):
                if tag == "a":
                    nc.scalar.copy(dst, src)
                else:
                    nc.vector.tensor_copy(dst, src)

            def gather(dst_ps, ent_col0, g_ops):
                for P in range(NPAIR):
                    nc.tensor.matmul(
                        dst_ps,
                        ent_sb[P][:, ent_col0:ent_col0 + GRP],
                        g_ops[P],
                        start=(P == 0),
                        stop=(P == NPAIR - 1),
                    )

            # ---- stage 1 interleaved with gathers: for each h-half, compute
            # ent for both pairs, evac, then immediately the O_j / S_i
            # gathers whose ent columns live in that half (slices 4h..4h+3),
            # two gathers per PSUM group tile -> one evac per slice-pair.
            ent_sb = []
            for P in range(NPAIR):
                esb = entsb.tile([128, H], BF16, tag=f"ent{P}")
                ent_sb.append(esb)
            o_sb = osb.tile([GRP, NJ, TP], BF16, tag="osb")
            s_sb = [None] * (NI // 2)
            # all ent matmuls first (PE churns while evacs trail), then the
            # O gather groups (o_sb completeness gates every multiply), then
            # S groups; S evacs go on ACT so DVE is free once mults start
            for half in range(2):
                for P in range(NPAIR):
                    eps = entps.tile([128, HHALF], F32, tag="eps")
                    for ck in range(CKP):
                        nc.tensor.matmul(
                            eps[:],
                            at_t[P][:, ck, :],
                            hid_t[P][:, ck, half * HHALF:(half + 1) * HHALF],
                            start=(ck == 0),
                            stop=(ck == CKP - 1),
                        )
                    evac(ENT_EVAC[half * NPAIR + P],
                         ent_sb[P][:, half * HHALF:(half + 1) * HHALF], eps[:])
            for pr in range(4):
                sl = 2 * pr
                ops = gps.tile([GRP, 2, SH], F32, tag="g")
                gather(ops[:, 0, 0:TP], sl * GRP, go_t)
                gather(ops[:, 1, 0:TP], (sl + 1) * GRP, go_t)
                evac(O_EVAC[pr], o_sb[:, sl:sl + 2, :], ops[:, :, 0:TP])
            for pr in range(4):
                sl = 2 * pr
                sps = gps.tile([GRP, 2, SH], F32, tag="g")
                gather(sps[:, 0, 0:TP], sl * GRP, gs_t)
                gather(sps[:, 1, 0:TP], (sl + 1) * GRP, gs_t)
                ssl = ssb.tile([GRP, 2, TP], BF16)
                nc.scalar.copy(ssl[:], sps[:, :, 0:TP])
                s_sb[pr] = ssl

            # ---- bl_i = S_i (x) O on DVE/Pool, then the final matmuls
            if final_t:
                lgt = []
                for k, (tc0, tcn) in enumerate(tcs):
                    lgtk = lgps.tile([tcn, NT], F32, tag=f"lg{k}")
                    lgt.append(lgtk)
            else:
                lg = lgps.tile([NT, TP], F32, tag="lg")
            for i in range(NI):
                bl = blp.tile([GRP, NJ, TP], BF16)
                mul_eng = nc.gpsimd if i in POOL_MULT else nc.vector
                mul_eng.tensor_mul(
                    bl[:],
                    s_sb[i // 2][:, i % 2, :].unsqueeze(1)
                        .broadcast_to((GRP, NJ, TP)),
                    o_sb[:],
                )
                for j in range(NJ):
                    if final_t:
                        for k, (tc0, tcn) in enumerate(tcs):
                            nc.tensor.matmul(
                                lgt[k][:],
                                bl[:, j, tc0:tc0 + tcn],
                                w_t[:, (i * NJ + j) * NT:(i * NJ + j + 1) * NT],
                                start=(i == 0 and j == 0),
                                stop=(i == NI - 1 and j == NJ - 1),
                            )
                    else:
                        nc.tensor.matmul(
                            lg[:],
                            w_t[:, (i * NJ + j) * NT:(i * NJ + j + 1) * NT],
                            bl[:, j, :],
                            start=(i == 0 and j == 0),
                            stop=(i == NI - 1 and j == NJ - 1),
                        )

            if final_t:
                for k, (tc0, tcn) in enumerate(tcs):
                    osb_k = outp.tile([tcn, NT], F32)
                    nc.scalar.copy(osb_k[:], lgt[k][:])
                    nc.sync.dma_start(out_d[k * 128:k * 128 + tcn, :], osb_k[:])
            else:
                out_sb = outp.tile([NT, TP], F32)
                nc.scalar.copy(out_sb[:], lg[:])
                nc.sync.dma_start(out_d[:], out_sb[:])

    nc.compile()
    return nc


def host_prep(plan, hidden_states, entity_subw_indices, entity_subw_mask,
              triplet_entity_nums, pair_mask, W_fc):
    """Build per-core input maps (numpy only)."""
    import ml_dtypes
    bf16 = ml_dtypes.bfloat16
    hs = np.asarray(hidden_states, dtype=np.float32)[:, :, HPERM].astype(bf16)
    idx = np.asarray(entity_subw_indices)
    msk = np.asarray(entity_subw_mask).astype(np.float32)
    trip = np.asarray(triplet_entity_nums)

    TP, RPP = plan.TP, plan.RPP
    cnt = np.maximum(msk.sum(axis=2), 1.0)          # (B, E)
    wgt = msk / cnt[:, :, None]                     # (B, E, S)

    w3 = (np.asarray(W_fc, np.float32)
          .reshape(GRP, NI * NJ * NT).astype(bf16))

    in_maps = []
    for c in range(NCORES):
        hid = np.zeros((NPAIR * RPP, H), bf16)
        at = np.zeros((NPAIR * RPP, 2 * E), np.float32)
        for P, (a, b2) in enumerate(plan.pairs[c]):
            off = P * RPP
            for local, ex in enumerate((a, b2)):
                rows = plan.rows[ex]
                hid[off:off + len(rows)] = hs[ex][rows]
                e_i, s_i = np.nonzero(msk[ex] > 0)
                l = idx[ex, e_i, s_i]
                r = off + np.searchsorted(rows, l)
                np.add.at(at, (r, local * E + e_i), wgt[ex, e_i, s_i])
                off += len(rows)
        g = np.zeros((2 * NPAIR, 128, TP), np.float32)
        for k, (b, t) in enumerate(plan.cols[c]):
            _, P, local = plan.exloc[b]
            g[NPAIR + P, local * E + trip[b, t, 0], k] = 1.0   # gs
            g[P, local * E + trip[b, t, 1], k] = 1.0           # go
        in_maps.append({
            "hid": hid,
            "at": at.astype(bf16),
            "g": g.astype(bf16),
            "w": w3,
        })
    return in_maps


def assemble(plan, results, b_fc):
    """results[c]["out"] is (NT, TP) -> (B, T, NT) + bias."""
    b_fc = np.asarray(b_fc, np.float32)
    logits = np.tile(b_fc[None, None, :], (B, T, 1))
    for c in range(NCORES):
        out = results[c]["out"]
        if out.shape[0] != NT:      # transposed t-chunk layout (final_t)
            chunks = []
            off = 0
            while off < plan.TP:
                n = min(128, plan.TP - off)
                k = off // 128
                chunks.append(out[k * 128:k * 128 + n, :])
                off += 128
            out = np.concatenate(chunks, axis=0).T
        if plan.cols[c]:
            bs = np.array([b for b, _ in plan.cols[c]])
            ts = np.array([t for _, t in plan.cols[c]])
            logits[bs, ts, :] = out[:, :len(bs)].T + b_fc[None, :]
    return logits


_NC_CACHE = {}


def kernel(hidden_states, entity_subw_indices, entity_subw_mask,
           triplet_entity_nums, pair_mask, W_fc, b_fc):
    plan = Plan(entity_subw_indices, entity_subw_mask,
                triplet_entity_nums, pair_mask)
    key = (plan.TP, plan.CKP, FINAL_T)
    if key not in _NC_CACHE:
        _NC_CACHE[key] = build_program(plan.TP, plan.CKP)
    nc = _NC_CACHE[key]
    in_maps = host_prep(plan, hidden_states, entity_subw_indices,
                        entity_subw_mask, triplet_entity_nums, pair_mask, W_fc)
    res = run_bass_kernel_spmd(nc, in_maps, core_ids=list(range(NCORES)))
    return assemble(plan, res.results, b_fc)


# revision 19
# speedup vs baseline: 1.7717x; 1.3063x over previous
"""Trainium2 Bass kernel for BilinearClassification (segment_reduce).

Math (per example b):
  ent[e,:]  = masked-mean over subword span of hidden[idx[e,s],:]      (E=64, H=768)
  subj[t,:] = ent[trip[t,0],:];  obj[t,:] = ent[trip[t,1],:]           (valid t only)
  bl[t, (g,i,j)] = subj[t, 8g+i] * obj[t, 8g+j]                        (f = 6144)
  logits[t,n] = bl[t,:] @ W[:,n] + b[n]                                (NT=42)

Device strategy (8 cores, data parallel, no collectives):
  - Host balances examples across cores by valid-triplet count, packs only the
    VALID triplet columns (T_PACK ~= 272 instead of 512) and only the USED
    hidden rows per example pair (RPP ~= 768 instead of 1024), and permutes
    hidden columns to (i,g) order: col' = i*96 + g for h = 8g+i.
  - stage 1: ent[e, (i,g)] = AT.T @ hid per pair (one-hot AT with mask/1/cnt
    folded), PSUM halves, ACT evac to bf16.
  - gathers: compact S_i[g,t] = ent[:, i*96:(i+1)*96].T @ G_s (accumulated over
    the core's two pairs), likewise O_j.  Thanks to the (i,g) column
    permutation these are plain 2D weight slices.  32 matmuls of T_PACK cols.
  - bl_i[g, j, t] = S_i[g, t] * O_j[g, t]: ONE broadcast-AP DVE multiply per i
    (all-SBUF bf16 -> 2x DVE rate).  No 6144-row expansion matmuls and no
    expanded-operand PSUM evacuations anywhere.
  - final: logits[n, t] += w3[:, i, j, :].T @ bl_i[:, j, :], 64 accumulating
    matmuls into one PSUM bank (42 x T_PACK).
  - host scatters packed columns back to (b, t) and adds b_fc.
"""
import sys

sys.path.insert(0, "/opt/trn_rl_repo")

import numpy as np

import concourse.bass as bass
import concourse.bacc as bacc
import concourse.tile as tile
from concourse import mybir
from concourse.bass_utils import run_bass_kernel_spmd

F32 = mybir.dt.float32
BF16 = mybir.dt.bfloat16

B, L, H = 32, 512, 768
E, S, T = 64, 8, 128
NT = 42
NCORES = 8
EXPC = B // NCORES          # 4 examples per core
NPAIR = EXPC // 2           # 2 example-pairs per core
GRP = 96                    # bilinear groups
NI = 8                      # i (subj) positions per group
NJ = 8                      # j (obj) positions per group
HHALF = H // 2

# hidden column permutation: col' = i*96 + g  <->  h = 8g + i
HPERM = np.empty(H, np.int64)
for _i in range(NI):
    for _g in range(GRP):
        HPERM[_i * GRP + _g] = 8 * _g + _i


class Plan:
    """Data-dependent packing decisions (shapes are compile-time params)."""

    def __init__(self, entity_subw_indices, entity_subw_mask,
                 triplet_entity_nums, pair_mask):
        idx = np.asarray(entity_subw_indices)
        msk = np.asarray(entity_subw_mask).astype(bool)
        pm = np.asarray(pair_mask).astype(bool)
        ntrip = pm.sum(axis=1).astype(int)

        # balance examples over cores by triplet count: greedy + swap
        # local search (TP <= 256 lets two gather tiles share a PSUM bank)
        order = np.argsort(-ntrip, kind="stable")
        cores = [[] for _ in range(NCORES)]
        loads = [0] * NCORES
        for ex in order:
            c = min((c for c in range(NCORES) if len(cores[c]) < EXPC),
                    key=lambda c: loads[c])
            cores[c].append(int(ex))
            loads[c] += int(ntrip[ex])
        for _ in range(1000):
            cmax = int(np.argmax(loads))
            best = None
            for c2 in range(NCORES):
                if c2 == cmax:
                    continue
                for ia, a in enumerate(cores[cmax]):
                    for ib, b in enumerate(cores[c2]):
                        d = int(ntrip[a]) - int(ntrip[b])
                        if d <= 0:
                            continue
                        nm = max(loads[cmax] - d, loads[c2] + d)
                        if nm < loads[cmax] and (best is None or nm < best[0]):
                            best = (nm, c2, ia, ib)
            if best is None:
                break
            _, c2, ia, ib = best
            a, b = cores[cmax][ia], cores[c2][ib]
            cores[cmax][ia], cores[c2][ib] = b, a
            d = int(ntrip[a]) - int(ntrip[b])
            loads[cmax] -= d
            loads[c2] += d
        self.cores = cores
        self.TP = max(16, -(-max(loads) // 16) * 16)

        # rows actually used per example; pair big-with-small inside a core
        self.rows = [np.unique(idx[b][msk[b]]) for b in range(B)]
        uniq = [len(r) for r in self.rows]
        self.pairs = []            # per core: [(exA, exB), (exC, exD)]
        rpp = 1
        for c in range(NCORES):
            exs = sorted(cores[c], key=lambda b: -uniq[b])
            ps = [(exs[0], exs[3]), (exs[1], exs[2])]
            self.pairs.append(ps)
            for a, b2 in ps:
                rpp = max(rpp, uniq[a] + uniq[b2])
        self.RPP = -(-rpp // 128) * 128
        self.CKP = self.RPP // 128

        # packed triplet columns per core: (example, orig t)
        self.cols = []
        for c in range(NCORES):
            cc = []
            for b in cores[c]:
                for t in np.nonzero(pm[b])[0]:
                    cc.append((b, int(t)))
            self.cols.append(cc)

        # example -> (pair index, local index) within its core
        self.exloc = {}
        for c in range(NCORES):
            for P, (a, b2) in enumerate(self.pairs[c]):
                self.exloc[a] = (c, P, 0)
                self.exloc[b2] = (c, P, 1)


import os as _os
# i-slices whose bl multiply runs on Pool instead of DVE
POOL_MULT = tuple(int(x) for x in _os.environ.get("KPOOL_MULT", "5,6,7").split(",") if x != "")
ENT_EVAC = ("v", "a", "v", "a")      # per (half, pair) evac engine
O_EVAC = ("a", "v", "a", "v")    # per O slice-pair (O01, O23, O45, O67)
# transposed final matmuls (stream W, bl stationary)
FINAL_T = _os.environ.get("KFINAL_T", "1") == "1"


def build_program(TP, CKP, reps=1, final_t=None):
    if final_t is None:
        final_t = FINAL_T
    RPP = CKP * 128
    nc = bacc.Bacc("TRN2", target_bir_lowering=False, debug=False)

    # t-chunks (<=128 wide) for the transposed final mode
    tcs = []
    off = 0
    while off < TP:
        tcs.append((off, min(128, TP - off)))
        off += 128

    hid_d = nc.dram_tensor("hid", (NPAIR * RPP, H), BF16, kind="ExternalInput")
    at_d = nc.dram_tensor("at", (NPAIR * RPP, 2 * E), BF16, kind="ExternalInput")
    # g: [go0, go1, gs0, gs1] stacked -> one DMA
    g_d = nc.dram_tensor("g", (2 * NPAIR, 128, TP), BF16, kind="ExternalInput")
    w_d = nc.dram_tensor("w", (GRP, NI * NJ * NT), BF16, kind="ExternalInput")
    if final_t:
        out_d = nc.dram_tensor("out", (len(tcs) * 128, NT), F32,
                               kind="ExternalOutput")
    else:
        out_d = nc.dram_tensor("out", (NT, TP), F32, kind="ExternalOutput")

    hid_r = hid_d[:].rearrange("(q p) h -> p q h", p=128)   # q = pair*CKP + ck
    at_r = at_d[:].rearrange("(q p) e -> p q e", p=128)
    g_r = g_d[:].rearrange("k p t -> p k t")

    # gather-psum group geometry: two [GRP, TP] gathers share a group tile;
    # SH is the f32 stride between them (one bank when 2*TP*4 <= 2KB)
    SH = TP if TP <= 256 else 512
    if TP <= 256:
        gps_bufs, eps_bufs = 3, 3
    else:
        gps_bufs, eps_bufs = 2, (1 if final_t else 2)

    with tile.TileContext(nc) as tc:
        with (
            tc.tile_pool(name="consts", bufs=1) as consts,
            tc.tile_pool(name="hidp", bufs=2) as hidp,
            tc.tile_pool(name="atp", bufs=2) as atp,
            tc.tile_pool(name="entps", bufs=eps_bufs, space="PSUM") as entps,
            tc.tile_pool(name="entsb", bufs=2) as entsb,
            tc.tile_pool(name="gps", bufs=gps_bufs, space="PSUM") as gps,
            tc.tile_pool(name="osb", bufs=1) as osb,
            tc.tile_pool(name="ssb", bufs=4) as ssb,
            tc.tile_pool(name="blp", bufs=NI) as blp,
            tc.tile_pool(name="lgps", bufs=1, space="PSUM") as lgps,
            tc.tile_pool(name="outp", bufs=2) as outp,
        ):
          for _rep in range(reps):
            # ---- input DMAs spread over the three DMA-capable queues
            # (SP, Pool/SWDGE, ACT).  The issuing queue is blocked for the
            # whole transfer, so hid is chunk-split across all three; ACT
            # only carries one small chunk per pair so it is free for evacs.
            hm = CKP // 2
            at_t, hid_t = [], []
            for P in range(NPAIR):
                att = atp.tile([128, CKP, 2 * E], BF16)
                at_t.append(att)
                ht = hidp.tile([128, CKP, H], BF16)
                hid_t.append(ht)
            nc.sync.dma_start(at_t[0][:], at_r[:, 0:CKP, :])
            nc.gpsimd.dma_start(at_t[1][:], at_r[:, CKP:2 * CKP, :])
            for ck in range(CKP):
                # round-robin SP / Pool / ACT, pair 0 chunks first
                for P in range(NPAIR):
                    eng = (nc.sync, nc.gpsimd, nc.scalar)[(ck * NPAIR + P) % 3]
                    eng.dma_start(hid_t[P][:, ck, :], hid_r[:, P * CKP + ck, :])
            g_t = consts.tile([128, 2 * NPAIR, TP], BF16, tag="g")
            nc.sync.dma_start(g_t[:], g_r[:])
            go_t = [g_t[:, 0, :], g_t[:, 1, :]]
            gs_t = [g_t[:, 2, :], g_t[:, 3, :]]
            w_t = consts.tile([GRP, NI * NJ * NT], BF16, tag="w")
            nc.sync.dma_start(w_t[:], w_d[:])

            def evac(tag, dst, src):
                if tag == "a":
                    nc.scalar.copy(dst, src)
                else:
                    nc.vector.tensor_copy(dst, src)

            def gather(dst_ps, ent_col0, g_ops):
                for P in range(NPAIR):
                    nc.tensor.matmul(
                        dst_ps,
                        ent_sb[P][:, ent_col0:ent_col0 + GRP],
                        g_ops[P],
                        start=(P == 0),
                        stop=(P == NPAIR - 1),
                    )

            # ---- stage 1 interleaved with gathers: for each h-half, compute
            # ent for both pairs, evac, then immediately the O_j / S_i
            # gathers whose ent columns live in that half (slices 4h..4h+3),
            # two gathers per PSUM group tile -> one evac per slice-pair.
            ent_sb = []
            for P in range(NPAIR):
                esb = entsb.tile([128, H], BF16, tag=f"ent{P}")
                ent_sb.append(esb)
            o_sb = osb.tile([GRP, NJ, TP], BF16, tag="osb")
            s_sb = [None] * (NI // 2)
            # all ent matmuls first (PE churns while evacs trail), then the
            # O gather groups (o_sb completeness gates every multiply), then
            # S groups; S evacs go on ACT so DVE is free once mults start
            for half in range(2):
                for P in range(NPAIR):
                    eps = entps.tile([128, HHALF], F32, tag="eps")
                    for ck in range(CKP):
                        nc.tensor.matmul(
                            eps[:],
                            at_t[P][:, ck, :],
                            hid_t[P][:, ck, half * HHALF:(half + 1) * HHALF],
                            start=(ck == 0),
                            stop=(ck == CKP - 1),
                        )
                    evac(ENT_EVAC[half * NPAIR + P],
                         ent_sb[P][:, half * HHALF:(half + 1) * HHALF], eps[:])
            for pr in range(4):
                sl = 2 * pr
                ops = gps.tile([GRP, 2, SH], F32, tag="g")
                gather(ops[:, 0, 0:TP], sl * GRP, go_t)
                gather(ops[:, 1, 0:TP], (sl + 1) * GRP, go_t)
                evac(O_EVAC[pr], o_sb[:, sl:sl + 2, :], ops[:, :, 0:TP])
            for pr in range(4):
                sl = 2 * pr
                sps = gps.tile([GRP, 2, SH], F32, tag="g")
                gather(sps[:, 0, 0:TP], sl * GRP, gs_t)
                gather(sps[:, 1, 0:TP], (sl + 1) * GRP, gs_t)
                ssl = ssb.tile([GRP, 2, TP], BF16)
                nc.scalar.copy(ssl[:], sps[:, :, 0:TP])
                s_sb[pr] = ssl

            # ---- bl_i = S_i (x) O on DVE/Pool, then the final matmuls
            if final_t:
                lgt = []
                for k, (tc0, tcn) in enumerate(tcs):
                    lgtk = lgps.tile([tcn, NT], F32, tag=f"lg{k}")
                    lgt.append(lgtk)
            else:
                lg = lgps.tile([NT, TP], F32, tag="lg")
            for i in range(NI):
                bl = blp.tile([GRP, NJ, TP], BF16)
                mul_eng = nc.gpsimd if i in POOL_MULT else nc.vector
                mul_eng.tensor_mul(
                    bl[:],
                    s_sb[i // 2][:, i % 2, :].unsqueeze(1)
                        .broadcast_to((GRP, NJ, TP)),
                    o_sb[:],
                )
                for j in range(NJ):
                    if final_t:
                        for k, (tc0, tcn) in enumerate(tcs):
                            nc.tensor.matmul(
                                lgt[k][:],
                                bl[:, j, tc0:tc0 + tcn],
                                w_t[:, (i * NJ + j) * NT:(i * NJ + j + 1) * NT],
                                start=(i == 0 and j == 0),
                                stop=(i == NI - 1 and j == NJ - 1),
                            )
                    else:
                        nc.tensor.matmul(
                            lg[:],
                            w_t[:, (i * NJ + j) * NT:(i * NJ + j + 1) * NT],
                            bl[:, j, :],
                            start=(i == 0 and j == 0),
                            stop=(i == NI - 1 and j == NJ - 1),
                        )

            if final_t:
                for k, (tc0, tcn) in enumerate(tcs):
                    osb_k = outp.tile([tcn, NT], F32)
                    nc.scalar.copy(osb_k[:], lgt[k][:])
                    nc.sync.dma_start(out_d[k * 128:k * 128 + tcn, :], osb_k[:])
            else:
                out_sb = outp.tile([NT, TP], F32)
                nc.scalar.copy(out_sb[:], lg[:])
                nc.sync.dma_start(out_d[:], out_sb[:])

    nc.compile()
    return nc


def host_prep(plan, hidden_states, entity_subw_indices, entity_subw_mask,
              triplet_entity_nums, pair_mask, W_fc):
    """Build per-core input maps (numpy only)."""
    import ml_dtypes
    bf16 = ml_dtypes.bfloat16
    hs = np.asarray(hidden_states, dtype=np.float32)[:, :, HPERM].astype(bf16)
    idx = np.asarray(entity_subw_indices)
    msk = np.asarray(entity_subw_mask).astype(np.float32)
    trip = np.asarray(triplet_entity_nums)

    TP, RPP = plan.TP, plan.RPP
    cnt = np.maximum(msk.sum(axis=2), 1.0)          # (B, E)
    wgt = msk / cnt[:, :, None]                     # (B, E, S)

    w3 = (np.asarray(W_fc, np.float32)
          .reshape(GRP, NI * NJ * NT).astype(bf16))

    in_maps = []
    for c in range(NCORES):
        hid = np.zeros((NPAIR * RPP, H), bf16)
        at = np.zeros((NPAIR * RPP, 2 * E), np.float32)
        for P, (a, b2) in enumerate(plan.pairs[c]):
            off = P * RPP
            for local, ex in enumerate((a, b2)):
                rows = plan.rows[ex]
                hid[off:off + len(rows)] = hs[ex][rows]
                e_i, s_i = np.nonzero(msk[ex] > 0)
                l = idx[ex, e_i, s_i]
                r = off + np.searchsorted(rows, l)
                np.add.at(at, (r, local * E + e_i), wgt[ex, e_i, s_i])
                off += len(rows)
        g = np.zeros((2 * NPAIR, 128, TP), np.float32)
        for k, (b, t) in enumerate(plan.cols[c]):
            _, P, local = plan.exloc[b]
            g[NPAIR + P, local * E + trip[b, t, 0], k] = 1.0   # gs
            g[P, local * E + trip[b, t, 1], k] = 1.0           # go
        in_maps.append({
            "hid": hid,
            "at": at.astype(bf16),
            "g": g.astype(bf16),
            "w": w3,
        })
    return in_maps


def assemble(plan, results, b_fc):
    """results[c]["out"] is (NT, TP) -> (B, T, NT) + bias."""
    b_fc = np.asarray(b_fc, np.float32)
    logits = np.tile(b_fc[None, None, :], (B, T, 1))
    for c in range(NCORES):
        out = results[c]["out"]
        if out.shape[0] != NT:      # transposed t-chunk layout (final_t)
            chunks = []
            off = 0
            while off < plan.TP:
                n = min(128, plan.TP - off)
                k = off // 128
                chunks.append(out[k * 128:k * 128 + n, :])
                off += 128
            out = np.concatenate(chunks, axis=0).T
        if plan.cols[c]:
            bs = np.array([b for b, _ in plan.cols[c]])
            ts = np.array([t for _, t in plan.cols[c]])
            logits[bs, ts, :] = out[:, :len(bs)].T + b_fc[None, :]
    return logits


_NC_CACHE = {}


def kernel(hidden_states, entity_subw_indices, entity_subw_mask,
           triplet_entity_nums, pair_mask, W_fc, b_fc):
    plan = Plan(entity_subw_indices, entity_subw_mask,
                triplet_entity_nums, pair_mask)
    key = (plan.TP, plan.CKP, FINAL_T)
    if key not in _NC_CACHE:
        _NC_CACHE[key] = build_program(plan.TP, plan.CKP)
    nc = _NC_CACHE[key]
    in_maps = host_prep(plan, hidden_states, entity_subw_indices,
                        entity_subw_mask, triplet_entity_nums, pair_mask, W_fc)
    res = run_bass_kernel_spmd(nc, in_maps, core_ids=list(range(NCORES)))
    return assemble(plan, res.results, b_fc)


# revision 21
# speedup vs baseline: 3.8603x; 2.1789x over previous
"""Trainium2 Bass kernel for BilinearClassification (segment_reduce).

Math (per example b):
  ent[e,:]  = masked-mean over subword span of hidden[idx[e,s],:]      (E=64, H=768)
  subj[t,:] = ent[trip[t,0],:];  obj[t,:] = ent[trip[t,1],:]           (valid t only)
  bl[t, (g,i,j)] = subj[t, 8g+i] * obj[t, 8g+j]                        (f = 6144)
  logits[t,n] = bl[t,:] @ W[:,n] + b[n]                                (NT=42)

Device strategy (8 cores, data parallel, no collectives):
  - Host balances examples across cores by valid-triplet count, packs only the
    VALID triplet columns (T_PACK ~= 272 instead of 512) and only the USED
    hidden rows per example pair (RPP ~= 768 instead of 1024), and permutes
    hidden columns to (i,g) order: col' = i*96 + g for h = 8g+i.
  - stage 1: ent[e, (i,g)] = AT.T @ hid per pair (one-hot AT with mask/1/cnt
    folded), PSUM halves, ACT evac to bf16.
  - gathers: compact S_i[g,t] = ent[:, i*96:(i+1)*96].T @ G_s (accumulated over
    the core's two pairs), likewise O_j.  Thanks to the (i,g) column
    permutation these are plain 2D weight slices.  32 matmuls of T_PACK cols.
  - bl_i[g, j, t] = S_i[g, t] * O_j[g, t]: ONE broadcast-AP DVE multiply per i
    (all-SBUF bf16 -> 2x DVE rate).  No 6144-row expansion matmuls and no
    expanded-operand PSUM evacuations anywhere.
  - final: logits[n, t] += w3[:, i, j, :].T @ bl_i[:, j, :], 64 accumulating
    matmuls into one PSUM bank (42 x T_PACK).
  - host scatters packed columns back to (b, t) and adds b_fc.
"""
import sys

sys.path.insert(0, "/opt/trn_rl_repo")

import numpy as np

import concourse.bass as bass
import concourse.bacc as bacc
import concourse.tile as tile
from concourse import mybir
from concourse.bass_utils import run_bass_kernel_spmd

F32 = mybir.dt.float32
BF16 = mybir.dt.bfloat16

B, L, H = 32, 512, 768
E, S, T = 64, 8, 128
NT = 42
NCORES = 8
EXPC = B // NCORES          # 4 examples per core
NPAIR = EXPC // 2           # 2 example-pairs per core
GRP = 96                    # bilinear groups
NI = 8                      # i (subj) positions per group
NJ = 8                      # j (obj) positions per group
HHALF = H // 2

# hidden column permutation: col' = i*96 + g  <->  h = 8g + i
HPERM = np.empty(H, np.int64)
for _i in range(NI):
    for _g in range(GRP):
        HPERM[_i * GRP + _g] = 8 * _g + _i


class Plan:
    """Data-dependent packing decisions (shapes are compile-time params)."""

    def __init__(self, entity_subw_indices, entity_subw_mask,
                 triplet_entity_nums, pair_mask):
        idx = np.asarray(entity_subw_indices)
        msk = np.asarray(entity_subw_mask).astype(bool)
        pm = np.asarray(pair_mask).astype(bool)
        ntrip = pm.sum(axis=1).astype(int)

        # balance examples over cores by triplet count: greedy + swap
        # local search (TP <= 256 lets two gather tiles share a PSUM bank)
        order = np.argsort(-ntrip, kind="stable")
        cores = [[] for _ in range(NCORES)]
        loads = [0] * NCORES
        for ex in order:
            c = min((c for c in range(NCORES) if len(cores[c]) < EXPC),
                    key=lambda c: loads[c])
            cores[c].append(int(ex))
            loads[c] += int(ntrip[ex])
        for _ in range(1000):
            cmax = int(np.argmax(loads))
            best = None
            for c2 in range(NCORES):
                if c2 == cmax:
                    continue
                for ia, a in enumerate(cores[cmax]):
                    for ib, b in enumerate(cores[c2]):
                        d = int(ntrip[a]) - int(ntrip[b])
                        if d <= 0:
                            continue
                        nm = max(loads[cmax] - d, loads[c2] + d)
                        if nm < loads[cmax] and (best is None or nm < best[0]):
                            best = (nm, c2, ia, ib)
            if best is None:
                break
            _, c2, ia, ib = best
            a, b = cores[cmax][ia], cores[c2][ib]
            cores[cmax][ia], cores[c2][ib] = b, a
            d = int(ntrip[a]) - int(ntrip[b])
            loads[cmax] -= d
            loads[c2] += d
        self.cores = cores
        self.TP = max(16, -(-max(loads) // 16) * 16)

        # rows actually used per example; pair big-with-small inside a core
        self.rows = [np.unique(idx[b][msk[b]]) for b in range(B)]
        uniq = [len(r) for r in self.rows]
        self.pairs = []            # per core: [(exA, exB), (exC, exD)]
        rpp = 1
        for c in range(NCORES):
            exs = sorted(cores[c], key=lambda b: -uniq[b])
            ps = [(exs[0], exs[3]), (exs[1], exs[2])]
            self.pairs.append(ps)
            for a, b2 in ps:
                rpp = max(rpp, uniq[a] + uniq[b2])
        self.RPP = -(-rpp // 128) * 128
        self.CKP = self.RPP // 128

        # packed triplet columns per core: (example, orig t)
        self.cols = []
        for c in range(NCORES):
            cc = []
            for b in cores[c]:
                for t in np.nonzero(pm[b])[0]:
                    cc.append((b, int(t)))
            self.cols.append(cc)

        # example -> (pair index, local index) within its core
        self.exloc = {}
        for c in range(NCORES):
            for P, (a, b2) in enumerate(self.pairs[c]):
                self.exloc[a] = (c, P, 0)
                self.exloc[b2] = (c, P, 1)


import os as _os
# i-slices whose bl multiply runs on Pool instead of DVE
POOL_MULT = tuple(int(x) for x in _os.environ.get("KPOOL_MULT", "").split(",") if x != "")
ENT_EVAC = tuple(_os.environ.get("KENT_EVAC", "a,a,a,a").split(","))
O_EVAC = tuple(_os.environ.get("KO_EVAC", "a,a,a,a").split(","))
# transposed final matmuls (stream W, bl stationary)
FINAL_T = _os.environ.get("KFINAL_T", "1") == "1"


def build_program(TP, CKP, reps=1, final_t=None):
    if final_t is None:
        final_t = FINAL_T
    RPP = CKP * 128
    nc = bacc.Bacc("TRN2", target_bir_lowering=False, debug=False)

    # t-chunks (<=128 wide) for the transposed final mode
    tcs = []
    off = 0
    while off < TP:
        tcs.append((off, min(128, TP - off)))
        off += 128

    hid_d = nc.dram_tensor("hid", (NPAIR * RPP, H), BF16, kind="ExternalInput")
    at_d = nc.dram_tensor("at", (NPAIR * RPP, 2 * E), BF16, kind="ExternalInput")
    # g: [go0, go1, gs0, gs1] stacked -> one DMA
    g_d = nc.dram_tensor("g", (2 * NPAIR, 128, TP), BF16, kind="ExternalInput")
    w_d = nc.dram_tensor("w", (GRP, NI * NJ * NT), BF16, kind="ExternalInput")
    if final_t:
        out_d = nc.dram_tensor("out", (len(tcs) * 128, NT), F32,
                               kind="ExternalOutput")
    else:
        out_d = nc.dram_tensor("out", (NT, TP), F32, kind="ExternalOutput")

    hid_r = hid_d[:].rearrange("(q p) h -> p q h", p=128)   # q = pair*CKP + ck
    at_r = at_d[:].rearrange("(q p) e -> p q e", p=128)
    g_r = g_d[:].rearrange("k p t -> p k t")

    # gather-psum group geometry: two [GRP, TP] gathers share a group tile;
    # SH is the f32 stride between them (one bank when 2*TP*4 <= 2KB)
    SH = TP if TP <= 256 else 512
    if TP <= 256:
        gps_bufs, eps_bufs = 3, 3
    else:
        gps_bufs, eps_bufs = 2, (1 if final_t else 2)

    with tile.TileContext(nc) as tc:
        with (
            tc.tile_pool(name="consts", bufs=1) as consts,
            tc.tile_pool(name="hidp", bufs=2) as hidp,
            tc.tile_pool(name="atp", bufs=2) as atp,
            tc.tile_pool(name="entps", bufs=eps_bufs, space="PSUM") as entps,
            tc.tile_pool(name="entsb", bufs=2) as entsb,
            tc.tile_pool(name="gps", bufs=gps_bufs, space="PSUM") as gps,
            tc.tile_pool(name="osb", bufs=2) as osb,
            tc.tile_pool(name="ssb", bufs=4) as ssb,
            tc.tile_pool(name="blp", bufs=NI) as blp,
            tc.tile_pool(name="lgps", bufs=1, space="PSUM") as lgps,
            tc.tile_pool(name="outp", bufs=2) as outp,
        ):
          for _rep in range(reps):
            # ---- input DMAs spread over the three DMA-capable queues
            # (SP, Pool/SWDGE, ACT).  The issuing queue is blocked for the
            # whole transfer, so hid is chunk-split across all three; ACT
            # only carries one small chunk per pair so it is free for evacs.
            hm = CKP // 2
            at_t, hid_t = [], []
            for P in range(NPAIR):
                att = atp.tile([128, CKP, 2 * E], BF16)
                at_t.append(att)
                ht = hidp.tile([128, CKP, H], BF16)
                hid_t.append(ht)
            nc.sync.dma_start(at_t[0][:], at_r[:, 0:CKP, :])
            nc.gpsimd.dma_start(at_t[1][:], at_r[:, CKP:2 * CKP, :])
            for ck in range(CKP):
                # round-robin SP / Pool / ACT, pair 0 chunks first
                for P in range(NPAIR):
                    eng = (nc.sync, nc.gpsimd, nc.scalar)[(ck * NPAIR + P) % 3]
                    eng.dma_start(hid_t[P][:, ck, :], hid_r[:, P * CKP + ck, :])
            g_t = consts.tile([128, 2 * NPAIR, TP], BF16, tag="g")
            nc.sync.dma_start(g_t[:], g_r[:])
            go_t = [g_t[:, 0, :], g_t[:, 1, :]]
            gs_t = [g_t[:, 2, :], g_t[:, 3, :]]
            w_t = consts.tile([GRP, NI * NJ * NT], BF16, tag="w")
            nc.sync.dma_start(w_t[:], w_d[:])

            def evac(tag, dst, src):
                if tag == "a":
                    nc.scalar.copy(dst, src)
                else:
                    nc.vector.tensor_copy(dst, src)

            def gather(dst_ps, ent_col0, g_ops):
                for P in range(NPAIR):
                    nc.tensor.matmul(
                        dst_ps,
                        ent_sb[P][:, ent_col0:ent_col0 + GRP],
                        g_ops[P],
                        start=(P == 0),
                        stop=(P == NPAIR - 1),
                    )

            # ---- stage 1 interleaved with gathers: for each h-half, compute
            # ent for both pairs, evac, then immediately the O_j / S_i
            # gathers whose ent columns live in that half (slices 4h..4h+3),
            # two gathers per PSUM group tile -> one evac per slice-pair.
            ent_sb = []
            for P in range(NPAIR):
                esb = entsb.tile([128, H], BF16, tag=f"ent{P}")
                ent_sb.append(esb)
            o_sb = osb.tile([GRP, NJ, TP], BF16, tag="osb")
            s_sb = [None] * (NI // 2)
            # all ent matmuls first (PE churns while evacs trail), then the
            # O gather groups (o_sb completeness gates every multiply), then
            # S groups; S evacs go on ACT so DVE is free once mults start
            for half in range(2):
                for P in range(NPAIR):
                    eps = entps.tile([128, HHALF], F32, tag="eps")
                    for ck in range(CKP):
                        nc.tensor.matmul(
                            eps[:],
                            at_t[P][:, ck, :],
                            hid_t[P][:, ck, half * HHALF:(half + 1) * HHALF],
                            start=(ck == 0),
                            stop=(ck == CKP - 1),
                        )
                    evac(ENT_EVAC[half * NPAIR + P],
                         ent_sb[P][:, half * HHALF:(half + 1) * HHALF], eps[:])
            for pr in range(4):
                sl = 2 * pr
                ops = gps.tile([GRP, 2, SH], F32, tag="g")
                gather(ops[:, 0, 0:TP], sl * GRP, go_t)
                gather(ops[:, 1, 0:TP], (sl + 1) * GRP, go_t)
                evac(O_EVAC[pr], o_sb[:, sl:sl + 2, :], ops[:, :, 0:TP])
            for pr in range(4):
                sl = 2 * pr
                sps = gps.tile([GRP, 2, SH], F32, tag="g")
                gather(sps[:, 0, 0:TP], sl * GRP, gs_t)
                gather(sps[:, 1, 0:TP], (sl + 1) * GRP, gs_t)
                ssl = ssb.tile([GRP, 2, TP], BF16)
                nc.scalar.copy(ssl[:], sps[:, :, 0:TP])
                s_sb[pr] = ssl

            # ---- bl_i = S_i (x) O on DVE/Pool, then the final matmuls
            if final_t:
                lgt = []
                for k, (tc0, tcn) in enumerate(tcs):
                    lgtk = lgps.tile([tcn, NT], F32, tag=f"lg{k}")
                    lgt.append(lgtk)
            else:
                lg = lgps.tile([NT, TP], F32, tag="lg")
            for i in range(NI):
                bl = blp.tile([GRP, NJ, TP], BF16)
                mul_eng = nc.gpsimd if i in POOL_MULT else nc.vector
                mul_eng.tensor_mul(
                    bl[:],
                    s_sb[i // 2][:, i % 2, :].unsqueeze(1)
                        .broadcast_to((GRP, NJ, TP)),
                    o_sb[:],
                )
                for j in range(NJ):
                    if final_t:
                        for k, (tc0, tcn) in enumerate(tcs):
                            nc.tensor.matmul(
                                lgt[k][:],
                                bl[:, j, tc0:tc0 + tcn],
                                w_t[:, (i * NJ + j) * NT:(i * NJ + j + 1) * NT],
                                start=(i == 0 and j == 0),
                                stop=(i == NI - 1 and j == NJ - 1),
                            )
                    else:
                        nc.tensor.matmul(
                            lg[:],
                            w_t[:, (i * NJ + j) * NT:(i * NJ + j + 1) * NT],
                            bl[:, j, :],
                            start=(i == 0 and j == 0),
                            stop=(i == NI - 1 and j == NJ - 1),
                        )

            if final_t:
                for k, (tc0, tcn) in enumerate(tcs):
                    osb_k = outp.tile([tcn, NT], F32)
                    nc.scalar.copy(osb_k[:], lgt[k][:])
                    nc.sync.dma_start(out_d[k * 128:k * 128 + tcn, :], osb_k[:])
            else:
                out_sb = outp.tile([NT, TP], F32)
                nc.scalar.copy(out_sb[:], lg[:])
                nc.sync.dma_start(out_d[:], out_sb[:])

    nc.compile()
    return nc


def host_prep(plan, hidden_states, entity_subw_indices, entity_subw_mask,
              triplet_entity_nums, pair_mask, W_fc):
    """Build per-core input maps (numpy only)."""
    import ml_dtypes
    bf16 = ml_dtypes.bfloat16
    hs = np.asarray(hidden_states, dtype=np.float32)[:, :, HPERM].astype(bf16)
    idx = np.asarray(entity_subw_indices)
    msk = np.asarray(entity_subw_mask).astype(np.float32)
    trip = np.asarray(triplet_entity_nums)

    TP, RPP = plan.TP, plan.RPP
    cnt = np.maximum(msk.sum(axis=2), 1.0)          # (B, E)
    wgt = msk / cnt[:, :, None]                     # (B, E, S)

    w3 = (np.asarray(W_fc, np.float32)
          .reshape(GRP, NI * NJ * NT).astype(bf16))

    in_maps = []
    for c in range(NCORES):
        hid = np.zeros((NPAIR * RPP, H), bf16)
        at = np.zeros((NPAIR * RPP, 2 * E), np.float32)
        for P, (a, b2) in enumerate(plan.pairs[c]):
            off = P * RPP
            for local, ex in enumerate((a, b2)):
                rows = plan.rows[ex]
                hid[off:off + len(rows)] = hs[ex][rows]
                e_i, s_i = np.nonzero(msk[ex] > 0)
                l = idx[ex, e_i, s_i]
                r = off + np.searchsorted(rows, l)
                np.add.at(at, (r, local * E + e_i), wgt[ex, e_i, s_i])
                off += len(rows)
        g = np.zeros((2 * NPAIR, 128, TP), np.float32)
        for k, (b, t) in enumerate(plan.cols[c]):
            _, P, local = plan.exloc[b]
            g[NPAIR + P, local * E + trip[b, t, 0], k] = 1.0   # gs
            g[P, local * E + trip[b, t, 1], k] = 1.0           # go
        in_maps.append({
            "hid": hid,
            "at": at.astype(bf16),
            "g": g.astype(bf16),
            "w": w3,
        })
    return in_maps


def assemble(plan, results, b_fc):
    """results[c]["out"] is (NT, TP) -> (B, T, NT) + bias."""
    b_fc = np.asarray(b_fc, np.float32)
    logits = np.tile(b_fc[None, None, :], (B, T, 1))
    for c in range(NCORES):
        out = results[c]["out"]
        if out.shape[0] != NT:      # transposed t-chunk layout (final_t)
            chunks = []
            off = 0
            while off < plan.TP:
                n = min(128, plan.TP - off)
                k = off // 128
                chunks.append(out[k * 128:k * 128 + n, :])
                off += 128
            out = np.concatenate(chunks, axis=0).T
        if plan.cols[c]:
            bs = np.array([b for b, _ in plan.cols[c]])
            ts = np.array([t for _, t in plan.cols[c]])
            logits[bs, ts, :] = out[:, :len(bs)].T + b_fc[None, :]
    return logits


_NC_CACHE = {}


def kernel(hidden_states, entity_subw_indices, entity_subw_mask,
           triplet_entity_nums, pair_mask, W_fc, b_fc):
    plan = Plan(entity_subw_indices, entity_subw_mask,
                triplet_entity_nums, pair_mask)
    key = (plan.TP, plan.CKP, FINAL_T)
    if key not in _NC_CACHE:
        _NC_CACHE[key] = build_program(plan.TP, plan.CKP)
    nc = _NC_CACHE[key]
    in_maps = host_prep(plan, hidden_states, entity_subw_indices,
                        entity_subw_mask, triplet_entity_nums, pair_mask, W_fc)
    res = run_bass_kernel_spmd(nc, in_maps, core_ids=list(range(NCORES)))
    return assemble(plan, res.results, b_fc)
